# revision 1
# baseline (speedup 1.0000x reference)
"""DistMult edge scoring on 8 Trainium2 NeuronCores.

score[e] = sum_d node_emb[src[e], d] * rel_emb[e, d] * node_emb[dst[e], d]

Strategy (data-parallel over edges, per the sharding hint):
  - Edges sharded contiguously across 8 cores (125k/core, padded to whole
    128x32 tiles). Node table replicated per core in HBM.
  - Gather: gpsimd dma_gather is descriptor-generation-bound on the Q7
    DSP cores (~8 ns/descriptor measured). The ucode gates descgen with
    `cpu_id / 2 == queue_num`, so gathers on different SWDGE queue_nums
    generate descriptors CONCURRENTLY on different Q7 core pairs. Each
    tile's two directions are split into four 2048-index gathers on
    queues 0-3 -> ~4x the descriptor throughput of the single-queue
    baseline (2.05 ms Q7-busy -> ~0.5 ms).
  - int16 gather indices only span 32768 values, but the Q7 descgen
    address math is SIGNED (IVP_MULUSAN_2X32: addr = base + stride*idx),
    so gathering from a mid-table base at group 25000 with
    idx = (node >> 1) - 25000 in [-25000, 25000) covers all 50000 2-row
    groups: one 512B descriptor per edge per direction, DVE selects the
    row with host-precomputed 0/1 masks (node & 1). The ucode trims
    TRAILING negative indices as pads, so the host swaps an edge with
    both endpoints >= 50000 (or a pad, which uses idx 0) into each
    sub-gather's last slot; scores are un-permuted on the host.
  - Raw bacc, manually pipelined: 4 gather slots, 4 prefetch slots.
    Engines settle at DVE ~660us / Q7 ~625us / DMA ~715us of ~745us.
"""

import numpy as np

import concourse.bacc as bacc
import concourse.bass as bass
import concourse.mybir as mybir
from concourse import library_config
from concourse.bass_utils import run_bass_kernel_spmd

N_NODES = 100000
DIM = 64
N_EDGES = 1000000
N_CORES = 8

P = 128
K = 32
TILE = P * K                      # 4096 edges per tile
EPC = N_EDGES // N_CORES          # 125000
NT = -(-EPC // TILE)              # 31
EPAD = NT * TILE                  # 126976

GR = 2                            # rows per gather group
NGRP = N_NODES // GR              # 50000 groups
GOFF = NGRP // 2                  # gather base at group 25000; idx is a
                                  # SIGNED int16 offset (the Q7 descgen uses
                                  # IVP_MULUSAN_2X32: address = base +
                                  # stride * signed(idx)), covering all
                                  # 50000 groups from the mid-table base
HALF = TILE // 2                  # 2048 idx per sub-gather
QH = HALF // P                    # 16 free slots per sub-gather
IDXH = HALF // 16                 # 128 int16s per partition per sub-gather

F32 = mybir.dt.float32

_cache = {}


def _half_valid(t, h):
    """Non-pad edges in tile t, half h (>=1 so the DMA completion fires;
    an all-pad half gathers just its idx-0 slot)."""
    lo = t * TILE + h * HALF
    return int(max(1, min(HALF, EPC - lo)))




def _build_program():
    if "nc" in _cache:
        return _cache["nc"]

    nc = bacc.Bacc(
        "TRN2",
        target_bir_lowering=False,
        debug=False,
        enable_asserts=False,
        num_devices=N_CORES,
        num_swdge_queues=4,
    )
    table = nc.dram_tensor("table", [NGRP, GR * DIM], F32, kind="ExternalInput")
    # idx[t, p, d, h, :]: sub-gather (d, h) of tile t, wrapped per 2048
    idx_h = nc.dram_tensor(
        "idx", [NT, P, 2, 2, IDXH], mybir.dt.int16, kind="ExternalInput"
    )
    rel_h = nc.dram_tensor("rel", [NT, P, K, DIM], F32, kind="ExternalInput")
    msk_h = nc.dram_tensor("msk", [NT, P, 2, GR, K], F32, kind="ExternalInput")
    out_h = nc.dram_tensor("score", [NT, P, K], F32, kind="ExternalOutput")

    NB = 4     # gather-buffer pipeline slots
    NB_IO = 4  # idx/rel/msk prefetch slots
    NOPS = 9   # DVE ops per tile

    from contextlib import ExitStack

    es = ExitStack()
    with es:
        block = es.enter_context(nc.Block())
        gbuf = es.enter_context(
            nc.sbuf_tensor("gbuf", [P, NB, 2, K, GR * DIM], F32))
        relb = es.enter_context(
            nc.sbuf_tensor("relb", [P, NB_IO, K, DIM], F32))
        mskb = es.enter_context(
            nc.sbuf_tensor("mskb", [P, NB_IO, 2, GR, K], F32))
        idxb = es.enter_context(
            nc.sbuf_tensor("idxb", [P, NB_IO, 2, 2, IDXH], mybir.dt.int16))
        hbuf = es.enter_context(nc.sbuf_tensor("hbuf", [P, K, DIM], F32))
        tbuf = es.enter_context(nc.sbuf_tensor("tbuf", [P, K, DIM], F32))
        tmpb = es.enter_context(nc.sbuf_tensor("tmpb", [P, K, DIM], F32))
        sb_s = es.enter_context(nc.sbuf_tensor("sbuf_s", [P, NB, K], F32))
        s_idx = es.enter_context(nc.semaphore("s_idx"))
        s_rel = es.enter_context(nc.semaphore("s_rel"))
        s_msk = es.enter_context(nc.semaphore("s_msk"))
        s_vc = es.enter_context(nc.semaphore("s_vc"))
        s_out = es.enter_context(nc.semaphore("s_out"))
        s_q = tuple(
            tuple(
                es.enter_context(nc.semaphore(f"s_q{q}{p}"))
                for p in ("a", "b")
            )
            for q in range(4)
        )

        def q_done(eng, q, t):
            # tile t's queue-q gather completed (parity sems: completions
            # on one sem are 2 tiles apart, so out-of-order DMA finishes
            # cannot be conflated)
            eng.wait_ge(s_q[q][t % 2], 16 * (t // 2 + 1))

        @block.sync
        def _(sp: bass.BassEngine):
            # pure prefetcher: never gated on the compute chain beyond
            # slot reuse (NB_IO slots deep)
            for t in range(NT):
                s = t % NB_IO
                if t >= 1:
                    # order completions: sem count N must imply tiles 0..N-1
                    # are actually resident (DMAs can finish out of order)
                    sp.wait_ge(s_idx, 16 * t)
                    sp.wait_ge(s_rel, 16 * t)
                    sp.wait_ge(s_msk, 16 * t)
                if t >= NB_IO:
                    # idx slot free once tile t-NB_IO's gathers retired
                    tt = t - NB_IO
                    for q in range(4):
                        q_done(sp, q, tt)
                sp.dma_start(out=idxb[:, s], in_=idx_h[t]).then_inc(s_idx, 16)
                if t >= NB_IO:
                    # rel/msk slots consumed by DVE of tile t-NB_IO
                    sp.wait_ge(s_vc, NOPS * (t - NB_IO + 1))
                sp.dma_start(out=relb[:, s], in_=rel_h[t]).then_inc(s_rel, 16)
                sp.dma_start(out=mskb[:, s], in_=msk_h[t]).then_inc(s_msk, 16)

        @block.scalar
        def _(sc: bass.BassEngine):
            # out-stores, decoupled from the prefetch stream
            for t in range(NT):
                sc.wait_ge(s_vc, NOPS * (t + 1))
                if t >= 1:
                    sc.wait_ge(s_out, 16 * t)
                sc.dma_start(
                    out=out_h[t], in_=sb_s[:, t % NB]
                ).then_inc(s_out, 16)
            sc.wait_ge(s_out, 16 * NT)

        @block.gpsimd
        def _(gp: bass.BassGpSimd):
            gp.load_library(library_config.mlp)
            for t in range(NT):
                s = t % NB
                gp.wait_ge(s_idx, 16 * (t + 1))
                if t >= NB:
                    # gather buffers of tile t-NB consumed by DVE
                    gp.wait_ge(s_vc, NOPS * (t - NB + 1))
                for d in range(2):
                    for h in range(2):
                        q = 2 * d + h
                        gp.dma_gather(
                            gbuf[:, s, d, h * QH : (h + 1) * QH],
                            table[GOFF:],
                            idxb[:, t % NB_IO, d, h],
                            HALF,
                            _half_valid(t, h),
                            GR * DIM,
                            elem_step=GR * DIM,
                            single_packet=False,
                            queue_num=q,
                        ).then_inc(s_q[q][t % 2], 16)

        @block.vector
        def _(v: bass.BassEngine):
            mult = mybir.AluOpType.mult
            add = mybir.AluOpType.add
            for t in range(NT):
                s = t % NB
                q_done(v, 0, t)  # src half 0 landed
                q_done(v, 1, t)  # src half 1 landed
                v.wait_ge(s_rel, 16 * (t + 1))
                v.wait_ge(s_msk, 16 * (t + 1))
                if t >= NB:
                    v.wait_ge(s_out, 16 * (t - NB + 1))
                if t >= 1:
                    # hbuf/tbuf/tmpb WAR vs previous tile's chain
                    v.wait_ge(s_vc, NOPS * t)
                # last tile: only ceil(valid/P) k-slots hold real edges
                KV = K if t < NT - 1 else -(-(EPC - t * TILE) // P)
                i = NOPS * t

                def op(instr):
                    nonlocal i
                    i += 1
                    instr.then_inc(s_vc, 1)

                def wait():
                    v.wait_ge(s_vc, i)

                for d, dst in ((0, hbuf), (1, tbuf)):
                    if d == 1:
                        q_done(v, 2, t)
                        q_done(v, 3, t)
                    g = gbuf[:, s, d]
                    for r in range(GR):
                        m = mskb[:, t % NB_IO, d, r, :KV].to_broadcast(
                            [P, KV, DIM]
                        )
                        gsl = g[:, :KV, r * DIM : (r + 1) * DIM]
                        if r == 0:
                            op(
                                v.tensor_tensor(
                                    out=dst[:, :KV], in0=gsl, in1=m, op=mult
                                )
                            )
                        else:
                            wait()
                            op(
                                v.tensor_tensor(
                                    out=tmpb[:, :KV], in0=gsl, in1=m, op=mult
                                )
                            )
                            wait()
                            op(
                                v.tensor_tensor(
                                    out=dst[:, :KV],
                                    in0=dst[:, :KV],
                                    in1=tmpb[:, :KV],
                                    op=add,
                                )
                            )
                wait()
                op(
                    v.tensor_tensor(
                        out=hbuf[:, :KV],
                        in0=hbuf[:, :KV],
                        in1=relb[:, t % NB_IO, :KV],
                        op=mult,
                    )
                )
                wait()
                op(
                    v.tensor_tensor(
                        out=hbuf[:, :KV], in0=hbuf[:, :KV], in1=tbuf[:, :KV],
                        op=mult,
                    )
                )
                wait()
                v.tensor_reduce(
                    out=sb_s[:, s, :KV],
                    in_=hbuf[:, :KV],
                    axis=mybir.AxisListType.X,
                    op=add,
                ).then_inc(s_vc, 1)

    nc.compile()
    _cache["nc"] = nc
    return nc


def _prep_idx(idx_global):
    """(EPAD,) node ids -> wrapped SIGNED int16 group offsets
    [NT, 2, P, IDXH]. offset = (node >> 1) - GOFF in [-25000, 25000);
    pad edges (node < 0) use offset 0 (a valid mid-table row; their
    garbage scores are dropped on the host)."""
    n = idx_global.reshape(NT, 2, HALF)
    g = np.where(n >= 0, (n >> 1) - GOFF, 0).astype(np.int16)
    # wrap: index j -> [j % 16, j // 16], replicated across 8 partition groups
    wr = g.reshape(NT, 2, IDXH, 16).swapaxes(2, 3)  # [NT, 2, 16, IDXH]
    return np.broadcast_to(
        wr[:, :, None, :, :], (NT, 2, 8, 16, IDXH)
    ).reshape(NT, 2, P, IDXH)


def _prep_msk(idx_global):
    """(EPAD,) node ids -> 0/1 row-select masks [NT, P, GR, K]."""
    sub = (np.maximum(idx_global.reshape(NT, K, P), 0) & 1).astype(np.int8)
    m = (sub[:, None, :, :] == np.arange(GR, dtype=np.int8)[None, :, None, None])
    # [NT, GR, K, P] -> [NT, P, GR, K]
    return np.ascontiguousarray(m.transpose(0, 3, 1, 2)).astype(np.float32)


def _shard_inputs(node_emb, rel_emb, src, dst):
    node_emb = np.asarray(node_emb, dtype=np.float32)
    rel_emb = np.asarray(rel_emb, dtype=np.float32)
    src = np.asarray(src, dtype=np.int64)
    dst = np.asarray(dst, dtype=np.int64)

    table = np.ascontiguousarray(node_emb.reshape(NGRP, GR * DIM))

    in_maps = []
    orders = []
    for c in range(N_CORES):
        sl = slice(c * EPC, (c + 1) * EPC)
        src_c = np.full(EPAD, -1, np.int64)
        dst_c = np.full(EPAD, -1, np.int64)
        rel_c = np.zeros((EPAD, DIM), np.float32)
        src_c[:EPC] = src[sl]
        dst_c[:EPC] = dst[sl]
        rel_c[:EPC] = rel_emb[sl]

        # Q7 descgen trims TRAILING negative offsets as pads, so each
        # sub-gather's last slot must hold non-negative offsets for BOTH
        # directions: node >= 2*GOFF for src and dst, or a pad slot.
        order = np.arange(EPAD)
        # only the first _half_valid() indices of each sub-gather are
        # processed (num_idxs_reg), so the trailing-negative trim sees the
        # LAST VALID slot: park an edge with both endpoints >= 2*GOFF there
        # (swapping within the valid prefix keeps every real edge gathered)
        safe = (src_c >= 2 * GOFF) & (dst_c >= 2 * GOFF)
        for seg0 in range(0, EPAD, HALF):
            nv = max(1, min(HALF, EPC - seg0))
            if EPC - seg0 <= 0:
                continue  # all-pad half: slot 0 has idx 0, non-negative
            seg = slice(seg0, seg0 + nv)
            if not safe[order[seg]][-1]:
                cand = np.nonzero(safe[order[seg]])[0]
                assert len(cand), "no safe last slot; would need host patch"
                j = seg0 + cand[0]
                order[[j, seg0 + nv - 1]] = order[[seg0 + nv - 1, j]]
        orders.append(order)
        src_c = src_c[order]
        dst_c = dst_c[order]
        rel_c = rel_c[order]

        # [NT, 2dir, 2half, P, IDXH] -> [NT, P, 2dir, 2half, IDXH]
        idx = np.stack(
            [_prep_idx(src_c), _prep_idx(dst_c)], axis=1
        ).transpose(0, 3, 1, 2, 4)
        msk = np.stack([_prep_msk(src_c), _prep_msk(dst_c)], axis=2)
        # [NT, P, 2, GR, K]
        # edge j at [p = j % 128, k = j // 128] -> rel[t, p, k]
        rel_t = np.ascontiguousarray(
            rel_c.reshape(NT, K, P, DIM).swapaxes(1, 2)
        )
        in_maps.append(
            {
                "table": table,
                "idx": np.ascontiguousarray(idx),
                "rel": rel_t,
                "msk": msk,
            }
        )
    return in_maps, orders


def run_on_hw(node_emb, rel_emb, src, dst, **spmd_kwargs):
    nc = _build_program()
    in_maps = _shard_inputs(node_emb, rel_emb, src, dst)
    res = run_bass_kernel_spmd(
        nc, in_maps[0], list(range(N_CORES)), **spmd_kwargs
    )
    in_maps, orders = in_maps
    parts = []
    for c in range(N_CORES):
        slot_scores = (
            np.asarray(res.results[c]["score"]).transpose(0, 2, 1).reshape(EPAD)
        )
        edge_scores = np.empty(EPAD, np.float32)
        edge_scores[orders[c]] = slot_scores
        parts.append(edge_scores[:EPC])
    return np.concatenate(parts), res


def kernel(node_emb, rel_emb, src, dst):
    scores, _ = run_on_hw(node_emb, rel_emb, src, dst)
    return scores



# revision 3
# speedup vs baseline: 1.5102x; 1.5102x over previous
"""DistMult edge scoring on 8 TRN2 cores — one-hot-matmul src gather.

score[e] = sum_d node_emb[src[e],d] * rel_emb[e,d] * node_emb[dst[e],d]

Structure (per core, src-range sharding: core c owns src in
[12500c, 12500(c+1))):
  - Edges grouped by src into NG=104 groups (consecutive node runs,
    <=128 nodes and <=TPB*128 edges each); each group = TPB=10 tiles of
    128 edges, dst-sorted within the group, pads spread evenly.
  - h (src row, full f32): PE one-hot gather. Stationary = fp8 one-hot
    [node, edge] (host stream); moving = resident bf16 hi/lo src-slice
    rows; two matmuls accumulate hi+lo into PSUM = exact-ish f32 h.
  - t (dst row): classic dma_gather of f32 rows (256B descriptors) with
    per-tile quantile bases so signed-int16 offsets always fit.
  - ACT drains PSUM->SBUF; DVE does u=t*rel, prod=u*h, reduce.
"""

import numpy as np

import concourse.bacc as bacc
import concourse.bass as bass
import concourse.mybir as mybir
from concourse import library_config
from concourse.bass_utils import run_bass_kernel_spmd

N_NODES = 100000
DIM = 64
N_EDGES = 1000000
N_CORES = 8
SLICE = N_NODES // N_CORES     # 12500 src nodes per core

NG = 104                       # groups per core
TPB = 10                       # tiles per group
SC = 8                         # groups per superchunk
NSC = NG // SC                 # 13 superchunks
TILE = 128
GEDGE = TPB * TILE             # 1280 edge slots per group
SLOTS = NG * GEDGE             # 133120 slots per core
BASES = [max(0, k * (N_NODES // TPB) - 8000) for k in range(TPB)]

F32 = mybir.dt.float32
BF16 = mybir.dt.bfloat16
FP8 = mybir.dt.float8e4
I16 = mybir.dt.int16

_cache = {}


def _build_program():
    if "nc" in _cache:
        return _cache["nc"]
    nc = bacc.Bacc(
        "TRN2",
        target_bir_lowering=False,
        debug=False,
        enable_asserts=False,
        num_devices=N_CORES,
        num_swdge_queues=4,
    )
    tbl = nc.dram_tensor("tbl", [N_NODES, DIM], F32, kind="ExternalInput")
    hsl_h = nc.dram_tensor("hsl", [128, NG, 2, DIM], BF16, kind="ExternalInput")
    oh_h = nc.dram_tensor("oh", [NSC, 128, SC, TPB, TILE], FP8, kind="ExternalInput")
    idx_h = nc.dram_tensor("idx", [NSC, 128, TPB, SC * TILE // 16], I16,
                           kind="ExternalInput")
    rel_h = nc.dram_tensor("rel", [NSC, 128, TPB, SC, DIM], F32,
                           kind="ExternalInput")
    out_h = nc.dram_tensor("score", [NSC, 128, TPB, SC], F32,
                           kind="ExternalOutput")

    from contextlib import ExitStack

    es = ExitStack()
    with es:
        block = es.enter_context(nc.Block())
        hsl = es.enter_context(nc.sbuf_tensor("hslb", [128, NG, 2, DIM], BF16))
        ohb = es.enter_context(
            nc.sbuf_tensor("ohb", [128, 2, SC, TPB, TILE], FP8))
        idxb = es.enter_context(
            nc.sbuf_tensor("idxb", [128, 2, TPB, SC * TILE // 16], I16))
        relb = es.enter_context(
            nc.sbuf_tensor("relb", [128, 2, TPB, SC, DIM], F32))
        gtb = es.enter_context(
            nc.sbuf_tensor("gtb", [128, 2, TPB, SC, DIM], F32))
        stb = es.enter_context(
            nc.sbuf_tensor("stb", [128, 2, TPB, SC, DIM], F32))
        scob = es.enter_context(nc.sbuf_tensor("scob", [128, 2, TPB, SC], F32))
        ps = es.enter_context(nc.psum_tensor("ps", [128, 2, 16, DIM], F32))

        s_hsl = es.enter_context(nc.semaphore("s_hsl"))
        s_oh = es.enter_context(nc.semaphore("s_oh"))
        s_idx = es.enter_context(nc.semaphore("s_idx"))
        s_rel = es.enter_context(nc.semaphore("s_rel"))
        s_mm = es.enter_context(nc.semaphore("s_mm"))     # per-group 2*TPB mms
        s_dr = es.enter_context(nc.semaphore("s_dr"))     # ACT drains (1/group)
        s_v = es.enter_context(nc.semaphore("s_v"))       # DVE ops (3/sc)
        s_out = es.enter_context(nc.semaphore("s_out"))
        sk = tuple(
            tuple(
                es.enter_context(nc.semaphore(f"sk{k}{p}")) for p in range(2)
            )
            for k in range(TPB)
        )

        MM = 2 * TPB  # matmuls per group

        @block.sync
        def _(sp: bass.BassEngine):
            sp.dma_start(out=hsl[:], in_=hsl_h[:]).then_inc(s_hsl, 16)
            for s in range(NSC):
                b = s % 2
                if s >= 1:
                    # order completions: count N implies chunks 0..N-1 resident
                    sp.wait_ge(s_idx, 16 * s)
                    sp.wait_ge(s_oh, 16 * s)
                    sp.wait_ge(s_rel, 16 * s)
                if s >= 2:
                    # oh/idx/rel buffer b free when superchunk s-2 consumed:
                    # oh consumed by PE (all mms of sc s-2 done)
                    sp.wait_ge(s_mm, MM * SC * (s - 1))
                    # idx consumed by gathers of sc s-2 (same parity as s)
                    for k in range(TPB):
                        sp.wait_ge(sk[k][s % 2], 16 * (s // 2))
                    # rel consumed by DVE of sc s-2
                    sp.wait_ge(s_v, 3 * (s - 1))
                sp.dma_start(out=idxb[:, b], in_=idx_h[s]).then_inc(s_idx, 16)
                sp.dma_start(out=ohb[:, b], in_=oh_h[s]).then_inc(s_oh, 16)
                sp.dma_start(out=relb[:, b], in_=rel_h[s]).then_inc(s_rel, 16)

        @block.gpsimd
        def _(gp: bass.BassGpSimd):
            gp.load_library(library_config.mlp)
            for s in range(NSC):
                b = s % 2
                gp.wait_ge(s_idx, 16 * (s + 1))
                if s >= 2:
                    # gt buffer b free when DVE consumed sc s-2 (reduce done)
                    gp.wait_ge(s_v, 3 * (s - 1))
                for k in range(TPB):
                    gp.dma_gather(
                        gtb[:, b, k],
                        tbl[BASES[k]:],
                        idxb[:, b, k],
                        SC * TILE,
                        SC * TILE,
                        DIM,
                        elem_step=DIM,
                        single_packet=False,
                        queue_num=k % 4,
                    ).then_inc(sk[k][s % 2], 16)

        @block.tensor
        def _(pe: bass.BassTensorEngine):
            pe.wait_ge(s_hsl, 16)
            for s in range(NSC):
                b = s % 2
                pe.wait_ge(s_oh, 16 * (s + 1))
                for g in range(SC):
                    gg = s * SC + g
                    pb = gg % 2
                    if gg >= 2:
                        # psum buffer pb free when group gg-2 drained
                        pe.wait_ge(s_dr, gg - 1)
                    for k in range(TPB):
                        oh_ap = ohb[:, b, g, k]
                        pe.matmul(
                            ps[:, pb, k],
                            oh_ap,
                            hsl[:, gg, 0],
                            start=True,
                            stop=False,
                        ).then_inc(s_mm, 1)
                        pe.matmul(
                            ps[:, pb, k],
                            oh_ap,
                            hsl[:, gg, 1],
                            start=False,
                            stop=True,
                        ).then_inc(s_mm, 1)

        @block.scalar
        def _(sc_e: bass.BassScalarEngine):
            for s in range(NSC):
                b = s % 2
                for g in range(SC):
                    gg = s * SC + g
                    pb = gg % 2
                    sc_e.wait_ge(s_mm, MM * (gg + 1))
                    if s >= 2:
                        # st buffer b free when DVE consumed sc s-2
                        sc_e.wait_ge(s_v, 3 * (s - 1))
                    # drain psum group gg -> st[:, b, :, g, :]
                    sc_e.copy(
                        out=stb[:, b, :, g], in_=ps[:, pb, :TPB]
                    ).then_inc(s_dr, 1)
                # out-store for sc s-2 after DVE finished it
                if s >= 2:
                    sc_e.wait_ge(s_v, 3 * (s - 1))
                    sc_e.wait_ge(s_out, 16 * (s - 2))
                    sc_e.dma_start(
                        out=out_h[s - 2], in_=scob[:, s % 2]
                    ).then_inc(s_out, 16)
            for s in (NSC - 2, NSC - 1):
                sc_e.wait_ge(s_v, 3 * (s + 1))
                sc_e.wait_ge(s_out, 16 * s)
                sc_e.dma_start(
                    out=out_h[s], in_=scob[:, s % 2]
                ).then_inc(s_out, 16)
            sc_e.wait_ge(s_out, 16 * NSC)

        @block.vector
        def _(v: bass.BassEngine):
            mult = mybir.AluOpType.mult
            for s in range(NSC):
                b = s % 2
                v.wait_ge(s_rel, 16 * (s + 1))
                # all gathers of this superchunk landed
                for k in range(TPB):
                    v.wait_ge(sk[k][s % 2], 16 * (s // 2 + 1))
                # all drains of this superchunk done
                v.wait_ge(s_dr, SC * (s + 1))
                if s >= 2:
                    # scob buffer free when out-store of sc s-2 done
                    v.wait_ge(s_out, 16 * (s - 1))
                # u = t * rel  (overwrite gt)
                v.tensor_tensor(
                    out=gtb[:, b], in0=gtb[:, b], in1=relb[:, b], op=mult
                ).then_inc(s_v, 1)
                # prod = u * h (overwrite gt)
                v.wait_ge(s_v, 3 * s + 1)
                v.tensor_tensor(
                    out=gtb[:, b], in0=gtb[:, b], in1=stb[:, b], op=mult
                ).then_inc(s_v, 1)
                # score = reduce_d prod
                v.wait_ge(s_v, 3 * s + 2)
                v.tensor_reduce(
                    out=scob[:, b],
                    in_=gtb[:, b],
                    axis=mybir.AxisListType.X,
                    op=mybir.AluOpType.add,
                ).then_inc(s_v, 1)

    nc.compile()
    _cache["nc"] = nc
    return nc


def _pack_groups(src_c, c):
    """Pack the core's src-slice nodes into NG consecutive runs with
    <=128 nodes and <=GEDGE edges each. Returns group start/end node ids
    and per-edge group assignment implicitly via node->group."""
    lo = c * SLICE
    deg = np.bincount(src_c - lo, minlength=SLICE)

    def greedy(cap):
        n2g = np.empty(SLICE, np.int32)
        g = 0
        cnt = 0
        nn = 0
        for n in range(SLICE):
            if nn >= 128 or (cnt + deg[n] > cap and cnt > 0):
                g += 1
                cnt = 0
                nn = 0
            n2g[n] = g
            cnt += deg[n]
            nn += 1
        return g + 1, n2g

    lo_cap, hi_cap = int(deg.max()), GEDGE
    # smallest cap gives most groups; find cap with exactly NG groups
    best = None
    lo_c, hi_c = 64, GEDGE
    while lo_c <= hi_c:
        mid = (lo_c + hi_c) // 2
        ngr, n2g = greedy(mid)
        if ngr > NG:
            lo_c = mid + 1
        else:
            best = (ngr, n2g, mid)
            hi_c = mid - 1
    assert best is not None, "cannot pack into NG groups"
    ngr, n2g, cap = best
    assert ngr <= NG and cap <= GEDGE, (ngr, cap)
    # per-group edge counts
    gcnt = np.bincount(n2g[src_c - lo], minlength=NG)
    assert gcnt.max() <= GEDGE, gcnt.max()
    return n2g


def _shard_inputs(node_emb, rel_emb, src, dst):
    import ml_dtypes

    node_emb = np.asarray(node_emb, dtype=np.float32)
    rel_emb = np.asarray(rel_emb, dtype=np.float32)
    src = np.asarray(src, dtype=np.int64)
    dst = np.asarray(dst, dtype=np.int64)

    hi_full = node_emb.astype(ml_dtypes.bfloat16)
    lo_full = (node_emb - hi_full.astype(np.float32)).astype(ml_dtypes.bfloat16)

    core_of = (src // SLICE).astype(np.int32)
    in_maps = []
    infos = []
    for c in range(N_CORES):
        eids = np.nonzero(core_of == c)[0]
        src_c = src[eids]
        dst_c = dst[eids]
        lo = c * SLICE
        n2g = _pack_groups(src_c, c)
        egrp = n2g[src_c - lo]

        # slot assignment: within group sort by dst, spread across TPB
        # tiles evenly, pad each tile to 128
        slot_of = np.full(SLOTS, -1, np.int64)  # slot -> edge id (global)
        order = np.lexsort((dst_c, egrp))
        gcnt = np.bincount(egrp, minlength=NG)
        pos = 0
        for g in range(NG):
            ge = order[pos:pos + gcnt[g]]
            pos += gcnt[g]
            eg = gcnt[g]
            if eg >= 640:
                bounds = np.ceil(
                    np.arange(TPB + 1) * eg / TPB
                ).astype(np.int64)
                tiles = [ge[bounds[k]:bounds[k + 1]] for k in range(TPB)]
            else:
                tk = np.minimum(dst_c[ge] // (N_NODES // TPB), TPB - 1)
                tiles = [ge[tk == k] for k in range(TPB)]
            for k in range(TPB):
                te = tiles[k]
                assert len(te) <= TILE, (g, k, len(te))
                base_slot = (g * TPB + k) * TILE
                slot_of[base_slot:base_slot + len(te)] = eids[te]

        filled = slot_of >= 0
        s_src = np.where(filled, src[np.maximum(slot_of, 0)], -1)
        s_dst = np.where(filled, dst[np.maximum(slot_of, 0)], -1)

        # one-hot fp8 [NG, TPB, 128n, 128e]
        nloc = np.where(filled, s_src - lo, 0)
        # group start node (local): first node with n2g==g
        gstart = np.zeros(NG, np.int64)
        idxs = np.nonzero(np.diff(np.concatenate([[-1], n2g])))[0]
        for gi, st in zip(n2g[idxs], idxs):
            gstart[gi] = st
        srow = nloc.reshape(NG, TPB, TILE) - gstart[:, None, None]
        oh = (
            (srow[:, :, None, :] == np.arange(128)[None, None, :, None])
            & filled.reshape(NG, TPB, TILE)[:, :, None, :]
        )
        assert ((srow >= 0) & (srow < 128) | ~filled.reshape(NG, TPB, TILE)).all()
        # -> [NSC, 128, SC, TPB, TILE]
        oh8 = (
            oh.reshape(NSC, SC, TPB, 128, TILE)
            .transpose(0, 3, 1, 2, 4)
            .astype(np.float32)
            .astype(ml_dtypes.float8_e4m3)
        )

        # hsl [128, NG, 2, DIM] bf16: partition p of group g = node gstart+p
        nidx = np.minimum(gstart[None, :] + np.arange(128)[:, None], SLICE - 1)
        valid = (gstart[None, :] + np.arange(128)[:, None]) < SLICE
        hsl = np.zeros((128, NG, 2, DIM), ml_dtypes.bfloat16)
        hsl[:, :, 0] = np.where(valid[:, :, None], hi_full[lo + nidx], 0)
        hsl[:, :, 1] = np.where(valid[:, :, None], lo_full[lo + nidx], 0)

        # gather idx int16 offsets [NSC, TPB, SC*TILE] wrapped
        # slot j within (s, k): j = g_local*128 + p -> edge slot
        # (g = s*SC+g_local, k, p)
        sdst = s_dst.reshape(NG, TPB, TILE)
        offs = np.where(
            sdst >= 0, sdst - np.array(BASES)[None, :, None], 0
        ).astype(np.int64)
        assert (offs >= 0).all() and (offs <= 32767).all(), (
            offs.min(), offs.max())
        # [NSC, SC, TPB, TILE] -> [NSC, TPB, SC, TILE] -> [NSC, TPB, SC*TILE]
        offs = (
            offs.reshape(NSC, SC, TPB, TILE)
            .transpose(0, 2, 1, 3)
            .reshape(NSC, TPB, SC * TILE)
            .astype(np.int16)
        )
        # wrap 16 + replicate to 128 partitions
        wr = offs.reshape(NSC, TPB, SC * TILE // 16, 16).swapaxes(2, 3)
        idx16 = np.broadcast_to(
            wr[:, None, :, :, :], (NSC, 8, TPB, 16, SC * TILE // 16)
        ).transpose(0, 1, 3, 2, 4).reshape(NSC, 128, TPB, SC * TILE // 16)

        # rel [NSC, 128, TPB, SC, DIM]: edge (g=s*SC+gl, k, p) -> [s, p, k, gl]
        rel_slot = np.where(
            filled[:, None], rel_emb[np.maximum(slot_of, 0)], 0.0
        ).astype(np.float32)
        rel_t = np.ascontiguousarray(
            rel_slot.reshape(NSC, SC, TPB, TILE, DIM).transpose(0, 3, 2, 1, 4)
        )

        in_maps.append(
            {
                "tbl": node_emb,
                "hsl": hsl,
                "oh": np.ascontiguousarray(oh8),
                "idx": np.ascontiguousarray(idx16),
                "rel": rel_t,
            }
        )
        infos.append(slot_of)
    return in_maps, infos


def run_on_hw(node_emb, rel_emb, src, dst, **spmd_kwargs):
    nc = _build_program()
    in_maps, infos = _shard_inputs(node_emb, rel_emb, src, dst)
    res = run_bass_kernel_spmd(nc, in_maps, list(range(N_CORES)), **spmd_kwargs)
    out = np.empty(N_EDGES, np.float32)
    for c in range(N_CORES):
        # score [NSC, 128, TPB, SC] -> slot (g=s*SC+gl, k, p)
        sc_arr = np.asarray(res.results[c]["score"])
        slot_scores = sc_arr.transpose(0, 3, 2, 1).reshape(SLOTS)
        # slot layout: (g * TPB + k) * TILE + p where g = s*SC+gl
        slot_of = infos[c]
        m = slot_of >= 0
        out[slot_of[m]] = slot_scores[m]
    return out, res


def kernel(node_emb, rel_emb, src, dst):
    scores, _ = run_on_hw(node_emb, rel_emb, src, dst)
    return scores


# revision 4
# speedup vs baseline: 1.5585x; 1.0320x over previous
"""DistMult edge scoring on 8 TRN2 cores — one-hot-matmul src gather.

score[e] = sum_d node_emb[src[e],d] * rel_emb[e,d] * node_emb[dst[e],d]

Structure (per core, src-range sharding: core c owns src in
[12500c, 12500(c+1))):
  - Edges grouped by src into NG=104 groups (consecutive node runs,
    <=128 nodes and <=TPB*128 edges each); each group = TPB=10 tiles of
    128 edges, dst-sorted within the group, pads spread evenly.
  - h (src row, full f32): PE one-hot gather. Stationary = fp8 one-hot
    [node, edge] (host stream); moving = resident bf16 hi/lo src-slice
    rows; two matmuls accumulate hi+lo into PSUM = exact-ish f32 h.
  - t (dst row): classic dma_gather of f32 rows (256B descriptors) with
    per-tile quantile bases so signed-int16 offsets always fit.
  - ACT drains PSUM->SBUF; DVE does u=t*rel, prod=u*h, reduce.
"""

import numpy as np

import concourse.bacc as bacc
import concourse.bass as bass
import concourse.mybir as mybir
from concourse import library_config
from concourse.bass_utils import run_bass_kernel_spmd

N_NODES = 100000
DIM = 64
N_EDGES = 1000000
N_CORES = 8
SLICE = N_NODES // N_CORES     # 12500 src nodes per core

NG = 104                       # groups per core
TPB = 10                       # tiles per group
SC = 8                         # groups per superchunk
NSC = NG // SC                 # 13 superchunks
TILE = 128
GEDGE = TPB * TILE             # 1280 edge slots per group
SLOTS = NG * GEDGE             # 133120 slots per core
BASES = [max(0, k * (N_NODES // TPB) - 8000) for k in range(TPB)]

F32 = mybir.dt.float32
BF16 = mybir.dt.bfloat16
FP8 = mybir.dt.float8e4
I16 = mybir.dt.int16

_cache = {}


def _build_program():
    if "nc" in _cache:
        return _cache["nc"]
    nc = bacc.Bacc(
        "TRN2",
        target_bir_lowering=False,
        debug=False,
        enable_asserts=False,
        num_devices=N_CORES,
        num_swdge_queues=4,
    )
    tbl = nc.dram_tensor("tbl", [N_NODES, DIM], F32, kind="ExternalInput")
    hsl_h = nc.dram_tensor("hsl", [128, NG, 2, DIM], BF16, kind="ExternalInput")
    oh_h = nc.dram_tensor("oh", [NSC, 128, SC, TPB, TILE], FP8, kind="ExternalInput")
    idx_h = nc.dram_tensor("idx", [NSC, 128, TPB, SC * TILE // 16], I16,
                           kind="ExternalInput")
    rel_h = nc.dram_tensor("rel", [NSC, 128, TPB, SC, DIM], F32,
                           kind="ExternalInput")
    out_h = nc.dram_tensor("score", [NSC, 128, TPB, SC], F32,
                           kind="ExternalOutput")

    from contextlib import ExitStack

    es = ExitStack()
    with es:
        block = es.enter_context(nc.Block())
        hsl = es.enter_context(nc.sbuf_tensor("hslb", [128, NG, 2, DIM], BF16))
        ohb = es.enter_context(
            nc.sbuf_tensor("ohb", [128, 2, SC, TPB, TILE], FP8))
        idxb = es.enter_context(
            nc.sbuf_tensor("idxb", [128, 2, TPB, SC * TILE // 16], I16))
        relb = es.enter_context(
            nc.sbuf_tensor("relb", [128, 2, TPB, SC, DIM], F32))
        gtb = es.enter_context(
            nc.sbuf_tensor("gtb", [128, 2, TPB, SC, DIM], F32))
        stb = es.enter_context(
            nc.sbuf_tensor("stb", [128, 2, TPB, SC, DIM], F32))
        scob = es.enter_context(nc.sbuf_tensor("scob", [128, 2, TPB, SC], F32))
        ps = es.enter_context(nc.psum_tensor("ps", [128, 2, 16, DIM], F32))

        s_hsl = es.enter_context(nc.semaphore("s_hsl"))
        s_oh = es.enter_context(nc.semaphore("s_oh"))
        s_idx = es.enter_context(nc.semaphore("s_idx"))
        s_rel = es.enter_context(nc.semaphore("s_rel"))
        s_mm = es.enter_context(nc.semaphore("s_mm"))     # per-group 2*TPB mms
        s_dr = es.enter_context(nc.semaphore("s_dr"))     # ACT drains (1/group)
        s_v = es.enter_context(nc.semaphore("s_v"))       # DVE ops (3/sc)
        s_out = es.enter_context(nc.semaphore("s_out"))
        sk = tuple(
            tuple(
                es.enter_context(nc.semaphore(f"sk{k}{p}")) for p in range(2)
            )
            for k in range(TPB)
        )

        MM = 2 * TPB  # matmuls per group

        @block.sync
        def _(sp: bass.BassEngine):
            sp.dma_start(out=hsl[:], in_=hsl_h[:]).then_inc(s_hsl, 16)
            for s in range(NSC):
                b = s % 2
                if s >= 1:
                    # order completions: count N implies chunks 0..N-1 resident
                    sp.wait_ge(s_idx, 16 * s)
                    sp.wait_ge(s_oh, 16 * s)
                    sp.wait_ge(s_rel, 16 * s)
                if s >= 2:
                    # oh/idx/rel buffer b free when superchunk s-2 consumed:
                    # oh consumed by PE (all mms of sc s-2 done)
                    sp.wait_ge(s_mm, MM * SC * (s - 1))
                    # idx consumed by gathers of sc s-2 (same parity as s)
                    for k in range(TPB):
                        sp.wait_ge(sk[k][s % 2], 16 * (s // 2))
                    # rel consumed by DVE of sc s-2
                    sp.wait_ge(s_v, 3 * (s - 1))
                sp.dma_start(out=idxb[:, b], in_=idx_h[s]).then_inc(s_idx, 16)
                sp.dma_start(out=ohb[:, b], in_=oh_h[s]).then_inc(s_oh, 16)
                sp.dma_start(out=relb[:, b], in_=rel_h[s]).then_inc(s_rel, 16)

        @block.gpsimd
        def _(gp: bass.BassGpSimd):
            gp.load_library(library_config.mlp)
            for s in range(NSC):
                b = s % 2
                gp.wait_ge(s_idx, 16 * (s + 1))
                if s >= 2:
                    # gt buffer b free when DVE consumed sc s-2 (reduce done)
                    gp.wait_ge(s_v, 3 * (s - 1))
                for k in range(TPB):
                    gp.dma_gather(
                        gtb[:, b, k],
                        tbl[BASES[k]:],
                        idxb[:, b, k],
                        SC * TILE,
                        SC * TILE,
                        DIM,
                        elem_step=DIM,
                        single_packet=False,
                        queue_num=(k + 2 * s) % 4,
                    ).then_inc(sk[k][s % 2], 16)

        @block.tensor
        def _(pe: bass.BassTensorEngine):
            pe.wait_ge(s_hsl, 16)
            for s in range(NSC):
                b = s % 2
                pe.wait_ge(s_oh, 16 * (s + 1))
                for g in range(SC):
                    gg = s * SC + g
                    pb = gg % 2
                    if gg >= 2:
                        # psum buffer pb free when group gg-2 drained
                        pe.wait_ge(s_dr, gg - 1)
                    for k in range(TPB):
                        oh_ap = ohb[:, b, g, k]
                        pe.matmul(
                            ps[:, pb, k],
                            oh_ap,
                            hsl[:, gg, 0],
                            start=True,
                            stop=False,
                        ).then_inc(s_mm, 1)
                        pe.matmul(
                            ps[:, pb, k],
                            oh_ap,
                            hsl[:, gg, 1],
                            start=False,
                            stop=True,
                        ).then_inc(s_mm, 1)

        @block.scalar
        def _(sc_e: bass.BassScalarEngine):
            for s in range(NSC):
                b = s % 2
                for g in range(SC):
                    gg = s * SC + g
                    pb = gg % 2
                    sc_e.wait_ge(s_mm, MM * (gg + 1))
                    if s >= 2:
                        # st buffer b free when DVE consumed sc s-2
                        sc_e.wait_ge(s_v, 3 * (s - 1))
                    # drain psum group gg -> st[:, b, :, g, :]
                    sc_e.copy(
                        out=stb[:, b, :, g], in_=ps[:, pb, :TPB]
                    ).then_inc(s_dr, 1)
                # out-store for sc s-2 after DVE finished it
                if s >= 2:
                    sc_e.wait_ge(s_v, 3 * (s - 1))
                    sc_e.wait_ge(s_out, 16 * (s - 2))
                    sc_e.dma_start(
                        out=out_h[s - 2], in_=scob[:, s % 2]
                    ).then_inc(s_out, 16)
            for s in (NSC - 2, NSC - 1):
                sc_e.wait_ge(s_v, 3 * (s + 1))
                sc_e.wait_ge(s_out, 16 * s)
                sc_e.dma_start(
                    out=out_h[s], in_=scob[:, s % 2]
                ).then_inc(s_out, 16)
            sc_e.wait_ge(s_out, 16 * NSC)

        @block.vector
        def _(v: bass.BassEngine):
            mult = mybir.AluOpType.mult
            for s in range(NSC):
                b = s % 2
                v.wait_ge(s_rel, 16 * (s + 1))
                # all gathers of this superchunk landed
                for k in range(TPB):
                    v.wait_ge(sk[k][s % 2], 16 * (s // 2 + 1))
                # all drains of this superchunk done
                v.wait_ge(s_dr, SC * (s + 1))
                if s >= 2:
                    # scob buffer free when out-store of sc s-2 done
                    v.wait_ge(s_out, 16 * (s - 1))
                # u = t * rel  (overwrite gt)
                v.tensor_tensor(
                    out=gtb[:, b], in0=gtb[:, b], in1=relb[:, b], op=mult
                ).then_inc(s_v, 1)
                # prod = u * h (overwrite gt)
                v.wait_ge(s_v, 3 * s + 1)
                v.tensor_tensor(
                    out=gtb[:, b], in0=gtb[:, b], in1=stb[:, b], op=mult
                ).then_inc(s_v, 1)
                # score = reduce_d prod
                v.wait_ge(s_v, 3 * s + 2)
                v.tensor_reduce(
                    out=scob[:, b],
                    in_=gtb[:, b],
                    axis=mybir.AxisListType.X,
                    op=mybir.AluOpType.add,
                ).then_inc(s_v, 1)

    nc.compile()
    _cache["nc"] = nc
    return nc


def _pack_groups(src_c, c):
    """Pack the core's src-slice nodes into NG consecutive runs with
    <=128 nodes and <=GEDGE edges each. Returns group start/end node ids
    and per-edge group assignment implicitly via node->group."""
    lo = c * SLICE
    deg = np.bincount(src_c - lo, minlength=SLICE)

    def greedy(cap):
        n2g = np.empty(SLICE, np.int32)
        g = 0
        cnt = 0
        nn = 0
        for n in range(SLICE):
            if nn >= 128 or (cnt + deg[n] > cap and cnt > 0):
                g += 1
                cnt = 0
                nn = 0
            n2g[n] = g
            cnt += deg[n]
            nn += 1
        return g + 1, n2g

    lo_cap, hi_cap = int(deg.max()), GEDGE
    # smallest cap gives most groups; find cap with exactly NG groups
    best = None
    lo_c, hi_c = 64, GEDGE
    while lo_c <= hi_c:
        mid = (lo_c + hi_c) // 2
        ngr, n2g = greedy(mid)
        if ngr > NG:
            lo_c = mid + 1
        else:
            best = (ngr, n2g, mid)
            hi_c = mid - 1
    assert best is not None, "cannot pack into NG groups"
    ngr, n2g, cap = best
    assert ngr <= NG and cap <= GEDGE, (ngr, cap)
    # per-group edge counts
    gcnt = np.bincount(n2g[src_c - lo], minlength=NG)
    assert gcnt.max() <= GEDGE, gcnt.max()
    return n2g


def _shard_inputs(node_emb, rel_emb, src, dst):
    import ml_dtypes

    node_emb = np.asarray(node_emb, dtype=np.float32)
    rel_emb = np.asarray(rel_emb, dtype=np.float32)
    src = np.asarray(src, dtype=np.int64)
    dst = np.asarray(dst, dtype=np.int64)

    hi_full = node_emb.astype(ml_dtypes.bfloat16)
    lo_full = (node_emb - hi_full.astype(np.float32)).astype(ml_dtypes.bfloat16)

    core_of = (src // SLICE).astype(np.int32)
    in_maps = []
    infos = []
    for c in range(N_CORES):
        eids = np.nonzero(core_of == c)[0]
        src_c = src[eids]
        dst_c = dst[eids]
        lo = c * SLICE
        n2g = _pack_groups(src_c, c)
        egrp = n2g[src_c - lo]

        # slot assignment: within group sort by dst, spread across TPB
        # tiles evenly, pad each tile to 128
        slot_of = np.full(SLOTS, -1, np.int64)  # slot -> edge id (global)
        order = np.lexsort((dst_c, egrp))
        gcnt = np.bincount(egrp, minlength=NG)
        pos = 0
        for g in range(NG):
            ge = order[pos:pos + gcnt[g]]
            pos += gcnt[g]
            eg = gcnt[g]
            if eg >= 640:
                bounds = np.ceil(
                    np.arange(TPB + 1) * eg / TPB
                ).astype(np.int64)
                tiles = [ge[bounds[k]:bounds[k + 1]] for k in range(TPB)]
            else:
                tk = np.minimum(dst_c[ge] // (N_NODES // TPB), TPB - 1)
                tiles = [ge[tk == k] for k in range(TPB)]
            for k in range(TPB):
                te = tiles[k]
                assert len(te) <= TILE, (g, k, len(te))
                base_slot = (g * TPB + k) * TILE
                slot_of[base_slot:base_slot + len(te)] = eids[te]

        filled = slot_of >= 0
        s_src = np.where(filled, src[np.maximum(slot_of, 0)], -1)
        s_dst = np.where(filled, dst[np.maximum(slot_of, 0)], -1)

        # one-hot fp8 [NG, TPB, 128n, 128e]
        nloc = np.where(filled, s_src - lo, 0)
        # group start node (local): first node with n2g==g
        gstart = np.zeros(NG, np.int64)
        idxs = np.nonzero(np.diff(np.concatenate([[-1], n2g])))[0]
        for gi, st in zip(n2g[idxs], idxs):
            gstart[gi] = st
        srow = nloc.reshape(NG, TPB, TILE) - gstart[:, None, None]
        oh = (
            (srow[:, :, None, :] == np.arange(128)[None, None, :, None])
            & filled.reshape(NG, TPB, TILE)[:, :, None, :]
        )
        assert ((srow >= 0) & (srow < 128) | ~filled.reshape(NG, TPB, TILE)).all()
        # -> [NSC, 128, SC, TPB, TILE]
        oh8 = (
            oh.reshape(NSC, SC, TPB, 128, TILE)
            .transpose(0, 3, 1, 2, 4)
            .astype(np.float32)
            .astype(ml_dtypes.float8_e4m3)
        )

        # hsl [128, NG, 2, DIM] bf16: partition p of group g = node gstart+p
        nidx = np.minimum(gstart[None, :] + np.arange(128)[:, None], SLICE - 1)
        valid = (gstart[None, :] + np.arange(128)[:, None]) < SLICE
        hsl = np.zeros((128, NG, 2, DIM), ml_dtypes.bfloat16)
        hsl[:, :, 0] = np.where(valid[:, :, None], hi_full[lo + nidx], 0)
        hsl[:, :, 1] = np.where(valid[:, :, None], lo_full[lo + nidx], 0)

        # gather idx int16 offsets [NSC, TPB, SC*TILE] wrapped
        # slot j within (s, k): j = g_local*128 + p -> edge slot
        # (g = s*SC+g_local, k, p)
        sdst = s_dst.reshape(NG, TPB, TILE)
        offs = np.where(
            sdst >= 0, sdst - np.array(BASES)[None, :, None], 0
        ).astype(np.int64)
        assert (offs >= 0).all() and (offs <= 32767).all(), (
            offs.min(), offs.max())
        # [NSC, SC, TPB, TILE] -> [NSC, TPB, SC, TILE] -> [NSC, TPB, SC*TILE]
        offs = (
            offs.reshape(NSC, SC, TPB, TILE)
            .transpose(0, 2, 1, 3)
            .reshape(NSC, TPB, SC * TILE)
            .astype(np.int16)
        )
        # wrap 16 + replicate to 128 partitions
        wr = offs.reshape(NSC, TPB, SC * TILE // 16, 16).swapaxes(2, 3)
        idx16 = np.broadcast_to(
            wr[:, None, :, :, :], (NSC, 8, TPB, 16, SC * TILE // 16)
        ).transpose(0, 1, 3, 2, 4).reshape(NSC, 128, TPB, SC * TILE // 16)

        # rel [NSC, 128, TPB, SC, DIM]: edge (g=s*SC+gl, k, p) -> [s, p, k, gl]
        rel_slot = np.where(
            filled[:, None], rel_emb[np.maximum(slot_of, 0)], 0.0
        ).astype(np.float32)
        rel_t = np.ascontiguousarray(
            rel_slot.reshape(NSC, SC, TPB, TILE, DIM).transpose(0, 3, 2, 1, 4)
        )

        in_maps.append(
            {
                "tbl": node_emb,
                "hsl": hsl,
                "oh": np.ascontiguousarray(oh8),
                "idx": np.ascontiguousarray(idx16),
                "rel": rel_t,
            }
        )
        infos.append(slot_of)
    return in_maps, infos


def run_on_hw(node_emb, rel_emb, src, dst, **spmd_kwargs):
    nc = _build_program()
    in_maps, infos = _shard_inputs(node_emb, rel_emb, src, dst)
    res = run_bass_kernel_spmd(nc, in_maps, list(range(N_CORES)), **spmd_kwargs)
    out = np.empty(N_EDGES, np.float32)
    for c in range(N_CORES):
        # score [NSC, 128, TPB, SC] -> slot (g=s*SC+gl, k, p)
        sc_arr = np.asarray(res.results[c]["score"])
        slot_scores = sc_arr.transpose(0, 3, 2, 1).reshape(SLOTS)
        # slot layout: (g * TPB + k) * TILE + p where g = s*SC+gl
        slot_of = infos[c]
        m = slot_of >= 0
        out[slot_of[m]] = slot_scores[m]
    return out, res


def kernel(node_emb, rel_emb, src, dst):
    scores, _ = run_on_hw(node_emb, rel_emb, src, dst)
    return scores
